# revision 15
# baseline (speedup 1.0000x reference)
"""AdaptiveInput (adaptive embedding) Bass kernel for 8 TRN2 NeuronCores.

Strategy: data-parallel over tokens. Host sorts the 32768 token ids into 9
(cluster, 32k-row-chunk) segments (chunking keeps gather indices in int16
range), deals each segment's tokens round-robin across the 8 cores (so all
cores share one static graph with per-segment capacity = ceil(L_s/8)), and
builds per-core int16 index arrays in the dma_gather wrapped layout.

Device (per core, identical SPMD graph):
  - gpsimd dma_gather (transpose=True, bf16) pulls each segment's embedding
    rows from DRAM into SBUF already transposed: [128 h-part, hc, cap_g].
  - TensorE: per 128-token tile, out[tok, d] = sum_h eT[h, tok] * wT[h, d],
    accumulated over h-chunks into PSUM ([m, 512] per bank).
  - scalar/vector engines copy PSUM -> SBUF (one 512-col bank each).
  - sync engine DMAs the [m, 1024] f32 tile to the DRAM output staging.

Host reassembles: per (core, segment) the first count rows map back to the
dealt token positions; padded rows are discarded.
"""

import numpy as np
import ml_dtypes

import concourse.bacc as bacc
import concourse.bass as bass
import concourse.mybir as mybir
from concourse import library_config
from concourse.bass_utils import run_bass_kernel_spmd
from contextlib import ExitStack

N_CLASSES = 250000
CUTOFFS = [0, 10000, 60000, 190000, N_CLASSES]
D = 1024
H = [1024, 256, 64, 16]        # true embedding dims per cluster
HPAD = [1024, 256, 128, 128]   # padded to 256B rows for dma_gather (bf16)
HC = [8, 2, 1, 1]              # h-chunks of 128 partitions
CHUNK = 32768                  # table chunk rows (int16 index range)
NCORES = 8
NPSUM = 4                      # psum tile rotation depth (4 x 2 banks = 8)
NOUT = 4                       # out_sb rotation depth
OUT_DTYPE = mybir.dt.float32
BF16 = ml_dtypes.bfloat16

# segment table: (cluster, base_row, rows) — static given CUTOFFS/CHUNK
SEGS = []
_SEG_START = []
for _c in range(4):
    _SEG_START.append(len(SEGS))
    _osz = CUTOFFS[_c + 1] - CUTOFFS[_c]
    for _k in range((_osz + CHUNK - 1) // CHUNK):
        SEGS.append((_c, _k * CHUNK, min(CHUNK, _osz - _k * CHUNK)))
_SEG_START = np.array(_SEG_START)

_graph_cache = {}


def _roundup(x, m):
    return (x + m - 1) // m * m


def _wrap_idxs(arr, cap_g):
    """int16 array [cap_g] -> dma_gather wrapped layout [128, cap_g//16]."""
    w16 = arr.reshape(cap_g // 16, 16).T  # [16, cols]
    return np.tile(w16, (8, 1))           # replicate to 128 partitions


def _build_graph(caps):
    """caps: tuple of per-segment capacity (0 = segment absent)."""
    cap_g = [(_roundup(c, 128) if c else 0) for c in caps]
    idx_cols = sum(g // 16 for g in cap_g)
    tot_rows = sum(caps)

    # tiles: (seg, cluster, tok0, m, rowoff)
    tiles = []
    seg_rowoff = []
    seg_coloff = []
    ro = 0
    co = 0
    for s, (cl, base, rows) in enumerate(SEGS):
        seg_rowoff.append(ro)
        seg_coloff.append(co)
        c = caps[s]
        t0 = 0
        while t0 < c:
            m = min(128, c - t0)
            tiles.append((s, cl, t0, m, ro + t0))
            t0 += m
        ro += c
        co += cap_g[s] // 16
    present = [s for s in range(len(SEGS)) if caps[s] > 0]
    # gather completion index per segment (order of issue on gpsimd)
    gidx = {s: i for i, s in enumerate(present)}

    nc = bacc.Bacc("TRN2", debug=False)
    idx_t = nc.dram_tensor("idx", [128, idx_cols], mybir.dt.int16,
                           kind="ExternalInput")
    emb_t = [nc.dram_tensor(f"emb{c}", [CUTOFFS[c + 1] - CUTOFFS[c], HPAD[c]],
                            mybir.dt.bfloat16, kind="ExternalInput")
             for c in range(4)]
    wt_t = [nc.dram_tensor(f"wt{c}", [HC[c] * 128, D], mybir.dt.bfloat16,
                           kind="ExternalInput") for c in range(4)]
    out_t = nc.dram_tensor("out", [tot_rows, D], OUT_DTYPE,
                           kind="ExternalOutput")

    n_wt = sum(HC)

    with ExitStack() as es:
        idx_sb = es.enter_context(
            nc.sbuf_tensor("idx_sb", [128, idx_cols], mybir.dt.int16))
        wt_sb = [es.enter_context(
            nc.sbuf_tensor(f"wt_sb{c}", [128, HC[c], D], mybir.dt.bfloat16))
            for c in range(4)]
        eT_sb = {}
        for s in present:
            cl = SEGS[s][0]
            eT_sb[s] = es.enter_context(
                nc.sbuf_tensor(f"eT{s}", [128, HC[cl], cap_g[s]],
                               mybir.dt.bfloat16))
        out_sb = [es.enter_context(
            nc.sbuf_tensor(f"out_sb{i}", [128, D], OUT_DTYPE))
            for i in range(NOUT)]
        psum = [es.enter_context(
            nc.psum_tensor(f"ps{i}", [128, D], mybir.dt.float32))
            for i in range(NPSUM)]

        # DMA completion increments arrive piecemeal (evt_accel), so a wait
        # on a DMA sem is only sound when its threshold equals 16x the total
        # DMAs issued on that sem so far -> per-segment and per-buffer sems.
        # Allocated raw (not context-managed): cleared+freed after the Block
        # so NEFF re-executions see zeroed semaphores.
        sem_idx = nc.alloc_semaphore("sem_idx")
        sem_w = nc.alloc_semaphore("sem_w")
        sem_gs = {s: nc.alloc_semaphore(f"sem_g{s}") for s in present}
        sem_mm = nc.alloc_semaphore("sem_mm")
        sem_cpa = nc.alloc_semaphore("sem_cpa")
        sem_cpb = nc.alloc_semaphore("sem_cpb")
        sem_outb = [nc.alloc_semaphore(f"sem_out{i}") for i in range(NOUT)]
        all_sems = ([sem_idx, sem_w, sem_mm, sem_cpa, sem_cpb]
                    + [sem_gs[s] for s in present] + sem_outb)

        bes = ExitStack()
        block = bes.enter_context(nc.Block())

        @block.sync
        def _(sp: bass.BassEngine):
            sp.dma_start(idx_sb[:], idx_t[:]).then_inc(sem_idx, 16)
            for c in range(4):
                for k in range(HC[c]):
                    sp.dma_start(
                        wt_sb[c][:, k, :], wt_t[c][k * 128:(k + 1) * 128, :]
                    ).then_inc(sem_w, 16)
            for j, (s, cl, t0, m, rowoff) in enumerate(tiles):
                sp.wait_ge(sem_cpa, j + 1)
                sp.wait_ge(sem_cpb, j + 1)
                sp.dma_start(
                    out_t[rowoff:rowoff + m, :], out_sb[j % NOUT][:m, :]
                ).then_inc(sem_outb[j % NOUT], 16)

        ntiles = len(tiles)
        outb_counts = [sum(1 for j in range(ntiles) if j % NOUT == i)
                       for i in range(NOUT)]

        @block.gpsimd
        def _(g: bass.BassGpSimd):
            g.load_library(library_config.mlp)
            g.wait_ge(sem_idx, 16)
            for s in present:
                cl, base, rows = SEGS[s]
                cg = cap_g[s]
                co = seg_coloff[s]
                g.dma_gather(
                    eT_sb[s][:],
                    emb_t[cl][base:base + rows, :],
                    idx_sb[:, co:co + cg // 16],
                    cg, cg, HPAD[cl],
                    transpose=True,
                ).then_inc(sem_gs[s], 16)

        @block.tensor
        def _(te: bass.BassTensorEngine):
            te.wait_ge(sem_w, 16 * n_wt)
            last_seg = -1
            for j, (s, cl, t0, m, rowoff) in enumerate(tiles):
                if s != last_seg:
                    te.wait_ge(sem_gs[s], 16)
                    last_seg = s
                if j >= NPSUM:
                    te.wait_ge(sem_cpa, j - NPSUM + 1)
                    te.wait_ge(sem_cpb, j - NPSUM + 1)
                ps = psum[j % NPSUM]
                for half in range(2):
                    for k in range(HC[cl]):
                        mm = te.matmul(
                            ps[:m, half * 512:(half + 1) * 512],
                            eT_sb[s][:, k, t0:t0 + m],
                            wt_sb[cl][:, k, half * 512:(half + 1) * 512],
                            start=(k == 0), stop=(k == HC[cl] - 1),
                        )
                mm.then_inc(sem_mm, 1)

        @block.scalar
        def _(sc: bass.BassScalarEngine):
            for j, (s, cl, t0, m, rowoff) in enumerate(tiles):
                sc.wait_ge(sem_mm, j + 1)
                if j >= NOUT:
                    sc.wait_ge(sem_outb[j % NOUT], 16 * (j // NOUT))
                sc.copy(
                    out_sb[j % NOUT][:m, 0:512], psum[j % NPSUM][:m, 0:512]
                ).then_inc(sem_cpa, 1)

        @block.vector
        def _(ve: bass.BassVectorEngine):
            for j, (s, cl, t0, m, rowoff) in enumerate(tiles):
                ve.wait_ge(sem_mm, j + 1)
                if j >= NOUT:
                    ve.wait_ge(sem_outb[j % NOUT], 16 * (j // NOUT))
                ve.tensor_copy(
                    out_sb[j % NOUT][:m, 512:1024],
                    psum[j % NPSUM][:m, 512:1024],
                ).then_inc(sem_cpb, 1)

        # Block exit: all-engine barrier + engine/DMA drains. Then zero our
        # semaphores so a re-execution of the loaded NEFF starts clean.
        bes.close()
        nc.clear_and_free_semaphores(all_sems)
        nc.all_engine_barrier()

    nc.compile()
    meta = dict(cap_g=cap_g, seg_rowoff=seg_rowoff, seg_coloff=seg_coloff,
                idx_cols=idx_cols, tot_rows=tot_rows, present=present)
    return nc, meta


def _prep_tables(head_emb, head_w, tail0_emb, tail0_w, tail1_emb, tail1_w,
                 tail2_emb, tail2_w):
    embs_in = [head_emb, tail0_emb, tail1_emb, tail2_emb]
    ws_in = [head_w, tail0_w, tail1_w, tail2_w]
    embs, wts = [], []
    for c in range(4):
        e = np.asarray(embs_in[c], np.float32)
        if HPAD[c] != H[c]:
            ep = np.zeros((e.shape[0], HPAD[c]), BF16)
            ep[:, :H[c]] = e.astype(BF16)
        else:
            ep = np.ascontiguousarray(e.astype(BF16))
        embs.append(ep)
        w = np.asarray(ws_in[c], np.float32)  # [D, h]
        wp = np.zeros((HC[c] * 128, D), BF16)
        wp[:H[c], :] = w.T.astype(BF16)
        wts.append(wp)
    return embs, wts


def kernel(input, head_emb, head_w, tail0_emb, tail0_w, tail1_emb, tail1_w,
           tail2_emb, tail2_w, _trace=False, _tmpdir=None):
    ids = np.asarray(input)
    out_dt = np.int64 if ids.dtype == np.int64 else ids.dtype
    ids = ids.astype(np.int64)
    N = ids.shape[0]

    cl = np.searchsorted(np.array(CUTOFFS[1:]), ids, side="right")
    local = ids - np.array(CUTOFFS)[cl]
    seg_id = _SEG_START[cl] + local // CHUNK
    within = (local % CHUNK).astype(np.int16)

    counts_g = np.bincount(seg_id, minlength=len(SEGS))
    bounds = np.concatenate([[0], np.cumsum(counts_g)])
    order = np.argsort(seg_id, kind="stable")

    caps = tuple(int((c + NCORES - 1) // NCORES) for c in counts_g)
    key = caps
    if key not in _graph_cache:
        _graph_cache[key] = _build_graph(caps)
    nc, meta = _graph_cache[key]
    cap_g = meta["cap_g"]

    # per-core idx arrays in wrapped layout
    idx_arr = [np.zeros((128, meta["idx_cols"]), np.int16)
               for _ in range(NCORES)]
    deal = {}  # (s) -> list of per-core token-position arrays
    for s in range(len(SEGS)):
        if caps[s] == 0:
            continue
        toks = order[bounds[s]:bounds[s + 1]]
        percore = [toks[c::NCORES] for c in range(NCORES)]
        deal[s] = percore
        co = meta["seg_coloff"][s]
        w = cap_g[s] // 16
        for c in range(NCORES):
            arr = np.zeros(cap_g[s], np.int16)
            arr[:len(percore[c])] = within[percore[c]]
            idx_arr[c][:, co:co + w] = _wrap_idxs(arr, cap_g[s])

    embs, wts = _prep_tables(head_emb, head_w, tail0_emb, tail0_w,
                             tail1_emb, tail1_w, tail2_emb, tail2_w)

    in_maps = []
    for c in range(NCORES):
        m = {"idx": idx_arr[c]}
        for i in range(4):
            m[f"emb{i}"] = embs[i]
            m[f"wt{i}"] = wts[i]
        in_maps.append(m)

    res = run_bass_kernel_spmd(nc, in_maps, core_ids=list(range(NCORES)),
                               trace=_trace, tmpdir=_tmpdir)

    out = np.empty((N, D), np.float32)
    for s in range(len(SEGS)):
        if caps[s] == 0:
            continue
        ro = meta["seg_rowoff"][s]
        for c in range(NCORES):
            tk = deal[s][c]
            if len(tk) == 0:
                continue
            rows = res.results[c]["out"][ro:ro + len(tk)]
            out[tk] = rows.astype(np.float32)
    kernel._last_exec_time_ns = res.exec_time_ns
    return out


if __name__ == "__main__":
    # tiny self-check of host-side index plumbing (no device)
    rng = np.random.default_rng(0)
    ids = rng.integers(0, N_CLASSES, size=32768)
    cl = np.searchsorted(np.array(CUTOFFS[1:]), ids, side="right")
    assert ((ids >= np.array(CUTOFFS)[cl]) & (ids < np.array(CUTOFFS)[cl + 1])).all()
    print("host-side checks OK")
